# revision 17
# baseline (speedup 1.0000x reference)
"""Trainium2 Bass kernel for single-head causal self-attention.

Problem: x[4,2048,1024], Wq/Wk/Wv[1024,1024] (torch Linear convention,
y = x @ W.T), causal softmax(QK^T * 1/sqrt(d)) @ V, fp32.

Sharding: 8 cores = 4 batches x 2 interleaved query-tile sets. Algebra:
  scores = X (Wq^T Wk) X^T = (X M) X^T        (K projection folded away)
  out    = P V = P (X Wv^T) = (P X) Wv^T      (V projection folded away)
so each core runs: XM projection for its 1024 queries, causal scores
against the resident X^T, exp (no max subtraction -- logits bounded ~2.5),
PX accumulation over the same causal trip structure, then (PX) Wv^T.
Row sums ride the tensor engine via a ones-vector matmul; the host
divides by them when scattering strips back.

The two cores of a batch need different causal structures; to keep one
SPMD program, queries are assigned in an interleaved pattern
(tiles {0,3,4,7,8,11,12,15} vs {1,2,5,6,9,10,13,14}) chosen so that each
score/PX trip's active column range [off, 512) and its 128-wide mask
block offset are IDENTICAL across both parities -- only the mask block
content (open / triangle / full) differs, and that is input data.
Matmuls run restricted to the active range, so fully-masked columns
cost (almost) no PE time.
"""
import sys
import numpy as np

for p in ("/opt/trn_rl_repo", "/root/.axon_site/_ro/trn_rl_repo"):
    if p not in sys.path:
        sys.path.append(p)

import concourse.bass as bass
import concourse.tile as tile
from concourse import mybir, bacc
from concourse.bass_utils import run_bass_kernel_spmd
from contextlib import ExitStack

BF16 = mybir.dt.bfloat16
F32 = mybir.dt.float32

B, S, D, DO = 4, 2048, 1024, 1024
QB = 512                 # local q block (matmul moving dim)
SCALE = float(1.0 / np.sqrt(np.float32(DO)))
MASK_NEG = -1.0e6        # additive mask pre-scale

# interleaved q-tile ownership: parity p owns ASSIGN[p] (tiles of 128 q)
ASSIGN = [[0, 3, 4, 7, 8, 11, 12, 15], [1, 2, 5, 6, 9, 10, 13, 14]]
NJS = {0: 8, 1: 16}      # key-tile trips per local q block


def _off(lq, j):
    """Active column offset of trip (lq, j); moving range is [off, 512)."""
    return 128 * (j // 2) if lq == 0 else 128 * (max(j - 8, 0) // 2)


def _slot(lq, j):
    """Mask slot index (0..15) or None when both parities are fully open."""
    if lq == 0:
        return j
    return 8 + (j - 8) if j >= 8 else None


_PROG_CACHE = {}


def _build_program():
    nc = bacc.Bacc("TRN2", target_bir_lowering=False, debug=False)
    xq_d = nc.dram_tensor("xq", [128, 8 * 1024], BF16, kind="ExternalInput").ap()
    wm_d = nc.dram_tensor("wm", [128, 8 * 1024], BF16, kind="ExternalInput").ap()
    xk_d = nc.dram_tensor("xk", [128, 8 * 2048], BF16, kind="ExternalInput").ap()
    xn_d = nc.dram_tensor("xn", [128, 16 * 1024], BF16, kind="ExternalInput").ap()
    wv_d = nc.dram_tensor("wv", [128, 8 * 1024], BF16, kind="ExternalInput").ap()
    mk_d = nc.dram_tensor("mk", [128, 16 * 128], F32, kind="ExternalInput").ap()
    ones_d = nc.dram_tensor("ones_in", [128, 1], BF16, kind="ExternalInput").ap()
    ot_d = nc.dram_tensor("ot", [8, 128, 2 * QB], F32, kind="ExternalOutput").ap()
    rr_d = nc.dram_tensor("rr", [1, 2 * QB], F32, kind="ExternalOutput").ap()

    with tile.TileContext(nc) as tc:
        with ExitStack() as ctx:
            pers = ctx.enter_context(tc.tile_pool(name="pers", bufs=1))
            ax_pool = ctx.enter_context(tc.tile_pool(name="ax_pool", bufs=2))
            stage = ctx.enter_context(tc.tile_pool(name="stage", bufs=4))
            mm_ps = ctx.enter_context(
                tc.tile_pool(name="mm_ps", bufs=5, space="PSUM"))
            ax_ps = ctx.enter_context(
                tc.tile_pool(name="ax_ps", bufs=3, space="PSUM"))

            # ---- input DMAs, one queue (SP), in consumption order ----
            xq = pers.tile([128, 8 * 1024], BF16, tag="xq", name="xq")
            wm = pers.tile([128, 8 * 1024], BF16, tag="wm", name="wm")
            # eighth-interleave the first quarter (fast PE start), then
            # quarter-interleave the rest
            # xq on the SP queue, wm on the Activation queue: both HWDGE
            # sequencers issue in parallel so the PE's first chunk starts
            # ~1.4us earlier and the 4MB prologue streams at device rate
            for q in range(2):
                sl = slice(q * 1024, (q + 1) * 1024)
                nc.sync.dma_start(xq[:, sl], xq_d[:, sl])
                nc.scalar.dma_start(wm[:, sl], wm_d[:, sl])
            for q in range(1, 4):
                sl = slice(q * 2048, (q + 1) * 2048)
                nc.sync.dma_start(xq[:, sl], xq_d[:, sl])
                nc.scalar.dma_start(wm[:, sl], wm_d[:, sl])
            ones = pers.tile([128, 1], BF16, tag="ones", name="ones")
            nc.sync.dma_start(ones[:], ones_d)
            mk = pers.tile([128, 16 * 128], F32, tag="mk", name="mk")
            nc.sync.dma_start(mk[:], mk_d)
            xk = pers.tile([128, 8 * 2048], BF16, tag="xk", name="xk")
            nc.sync.dma_start(xk[:], xk_d)
            xn = pers.tile([128, 16 * 1024], BF16, tag="xn", name="xn")
            nc.sync.dma_start(xn[:], xn_d)
            wv = pers.tile([128, 8 * 1024], BF16, tag="wv", name="wv")
            nc.sync.dma_start(wv[:], wv_d)

            # ---- XM projection: qt[lq][t] = (M^T X^T)[t-tile, lq cols] ----
            # chunks of 2 t-values x 2 lq, dt-interleaved, to overlap with
            # the xq/wm quarter DMAs.
            qts = {}
            all_groups = [(t, lq) for t in range(8) for lq in range(2)]
            chunks = [all_groups[0:5], all_groups[5:10],
                      all_groups[10:15], all_groups[15:16]]
            for groups in chunks:
                ps = {}
                for (t, lq) in groups:
                    ps[(t, lq)] = mm_ps.tile(
                        [128, QB], F32, tag="ps", name=f"psq{t}_{lq}")
                for dt in range(8):
                    for (t, lq) in groups:
                        nc.tensor.matmul(
                            ps[(t, lq)][:],
                            wm[:, dt * 1024 + t * 128:dt * 1024 + (t + 1) * 128],
                            xq[:, dt * 1024 + lq * QB:dt * 1024 + (lq + 1) * QB],
                            start=(dt == 0), stop=(dt == 7))
                for (t, lq) in sorted(groups, key=lambda g: g[1]):
                    qt = pers.tile([128, QB], BF16, tag=f"qt{lq}_{t}",
                                   name=f"qt{lq}_{t}")
                    nc.scalar.copy(qt[:], ps[(t, lq)][:])
                    qts[(lq, t)] = qt

            # ---- attention per local q block ----
            for lq in range(2):
                njs = NJS[lq]
                Ps = []
                for j in range(njs):
                    o = _off(lq, j)
                    ps = mm_ps.tile([128, QB], F32, tag="ps",
                                    name=f"pss{lq}_{j}")
                    for t in range(8):
                        nc.tensor.matmul(
                            ps[:, o:QB],
                            xk[:, t * 2048 + j * 128:t * 2048 + (j + 1) * 128],
                            qts[(lq, t)][:, o:QB],
                            start=(t == 0), stop=(t == 7))
                    s = _slot(lq, j)
                    if s is not None:
                        nc.vector.tensor_add(
                            ps[:, o:o + 128], ps[:, o:o + 128],
                            mk[:, s * 128:(s + 1) * 128])
                    P = pers.tile([128, QB], BF16, tag=f"P{lq}_{j}",
                                  name=f"P{lq}_{j}")
                    nc.scalar.activation(
                        P[:, o:QB], ps[:, o:QB],
                        mybir.ActivationFunctionType.Exp, scale=SCALE)
                    Ps.append((P, o))

                # PX: ax[d] = sum_j xn[j,d]^T P[j]
                axs = []
                for d in range(8):
                    aps = ax_ps.tile([128, QB], F32, tag="aps",
                                     name=f"aps{lq}_{d}")
                    for j in range(njs):
                        P, o = Ps[j]
                        nc.tensor.matmul(
                            aps[:, o:QB],
                            xn[:, j * 1024 + d * 128:j * 1024 + (d + 1) * 128],
                            P[:, o:QB],
                            start=(j == 0), stop=(j == njs - 1),
                            skip_group_check=True)
                    ax = ax_pool.tile([128, QB], BF16, tag=f"ax{d}",
                                      name=f"ax{lq}_{d}")
                    nc.vector.tensor_copy(ax[:], aps[:])
                    axs.append(ax)

                # row sums after AX: P tiles are long since ready, so the
                # PE never stalls on the exp chain here
                r_psum = mm_ps.tile([1, QB], F32, tag="ps", name=f"r{lq}")
                for j in range(njs):
                    P, o = Ps[j]
                    nc.tensor.matmul(r_psum[:1, o:QB], ones[:], P[:, o:QB],
                                     start=(j == 0), stop=(j == njs - 1),
                                     skip_group_check=True)
                r_sb = stage.tile([1, QB], F32, tag="rsb", name=f"rsb{lq}",
                                  bufs=2)
                nc.vector.tensor_copy(r_sb[:1], r_psum[:1])
                nc.gpsimd.dma_start(rr_d[:, lq * QB:(lq + 1) * QB], r_sb[:1])

                # (PX) Wv^T per o-tile; final o-tile sliced to shorten the
                # trailing copy+store chain
                for ot in range(8):
                    last = (lq == 1 and ot == 7)
                    slices = ([(0, QB)] if not last
                              else [(c, c + 128) for c in range(0, QB, 128)])
                    for (c0, c1) in slices:
                        po = mm_ps.tile([128, c1 - c0], F32, tag="ps",
                                        name=f"po{lq}_{ot}_{c0}")
                        for d in range(8):
                            nc.tensor.matmul(
                                po[:],
                                wv[:, d * 1024 + ot * 128:d * 1024 + (ot + 1) * 128],
                                axs[d][:, c0:c1],
                                start=(d == 0), stop=(d == 7))
                        st = stage.tile([128, c1 - c0], F32, tag="st",
                                        name=f"st{lq}_{ot}_{c0}", bufs=8)
                        nc.scalar.copy(st[:], po[:])
                        # last-tile slices ride the idle HWDGE (SP) queue so
                        # the final store chain is short
                        eng = nc.sync if last else nc.gpsimd
                        eng.dma_start(
                            ot_d[ot][:, lq * QB + c0:lq * QB + c1], st[:])
    nc.compile()
    return nc


def _get_program():
    if "nc" not in _PROG_CACHE:
        _PROG_CACHE["nc"] = _build_program()
    return _PROG_CACHE["nc"]


def _make_mk(par):
    """Per-parity mask blocks [128, 16*128] f32, plus structure asserts."""
    mk = np.zeros((128, 16 * 128), np.float32)
    kk = np.arange(128)[:, None]
    cc = np.arange(128)[None, :]
    for lq in (0, 1):
        tiles = ASSIGN[par][4 * lq:4 * lq + 4]
        for j in range(NJS[lq]):
            o = _off(lq, j)
            s = _slot(lq, j)
            for sub in range(4):
                t = tiles[sub]
                lo = sub * 128
                if lo < o:
                    # excluded from the moving range: must be fully masked
                    assert t < j, (par, lq, j, sub)
                elif lo == o and s is not None:
                    # the mask block: 0 where key 128j+k <= query 128t+c
                    blk = np.where(128 * j + kk <= 128 * t + cc,
                                   0.0, MASK_NEG).astype(np.float32)
                    mk[:, s * 128:(s + 1) * 128] = blk
                else:
                    # inside the range but no mask applied: must be open
                    assert t > j, (par, lq, j, sub)
    return mk


def _make_in_maps(x, Wq, Wk, Wv):
    import ml_dtypes
    bf = ml_dtypes.bfloat16
    # fold both score projections into M = Wq^T Wk (host, fp32)
    M = np.ascontiguousarray(Wq.T.astype(np.float32) @ Wk.astype(np.float32))
    wm_h = np.ascontiguousarray(
        M.reshape(8, 128, 1024).transpose(1, 0, 2).reshape(128, 8192)
    ).astype(bf)
    wv_h = np.ascontiguousarray(
        Wv.T.reshape(8, 128, 1024).transpose(1, 0, 2).reshape(128, 8192)
    ).astype(bf)
    mks = [_make_mk(0), _make_mk(1)]
    ones_in = np.ones((128, 1), bf)

    in_maps = []
    for b in range(B):
        xb = np.asarray(x[b], dtype=np.float32)
        xT = xb.T                                    # [D, S]
        xk_h = np.ascontiguousarray(
            xT.reshape(8, 128, S).transpose(1, 0, 2).reshape(128, 8 * S)
        ).astype(bf)
        xn_h = np.ascontiguousarray(
            xb.reshape(16, 128, D).transpose(1, 0, 2).reshape(128, 16 * D)
        ).astype(bf)
        for par in range(2):
            qcols = np.concatenate(
                [xT[:, t * 128:(t + 1) * 128] for t in ASSIGN[par]], axis=1)
            xq_h = np.ascontiguousarray(
                qcols.reshape(8, 128, 1024).transpose(1, 0, 2).reshape(128, 8192)
            ).astype(bf)
            in_maps.append({
                "xq": xq_h, "wm": wm_h, "xk": xk_h, "xn": xn_h,
                "wv": wv_h, "mk": mks[par], "ones_in": ones_in,
            })
    return in_maps


def kernel(x, Wq, Wk, Wv):
    x = np.asarray(x, dtype=np.float32)
    Wq = np.asarray(Wq, dtype=np.float32)
    Wk = np.asarray(Wk, dtype=np.float32)
    Wv = np.asarray(Wv, dtype=np.float32)
    nc = _get_program()
    in_maps = _make_in_maps(x, Wq, Wk, Wv)
    res = run_bass_kernel_spmd(nc, in_maps, core_ids=list(range(8)))
    out = np.empty((B, S, DO), np.float32)
    out_g = out.reshape(B, 16, 128, DO)
    for b in range(B):
        for par in range(2):
            r = res.results[2 * b + par]
            ot = r["ot"].reshape(DO, 1024)       # [o, local q]
            rr = r["rr"][0]                      # [1024]
            vals = (ot / rr[None, :]).T          # [local q, o]
            out_g[b, ASSIGN[par]] = vals.reshape(8, 128, DO)
    return out


if __name__ == "__main__":
    rng = np.random.default_rng(0)
    x = rng.standard_normal((B, S, D)).astype(np.float32)
    Wq = (rng.standard_normal((DO, D)) * 0.02).astype(np.float32)
    Wk = (rng.standard_normal((DO, D)) * 0.02).astype(np.float32)
    Wv = (rng.standard_normal((DO, D)) * 0.02).astype(np.float32)
    out = kernel(x=x, Wq=Wq, Wk=Wk, Wv=Wv)
    print("out", out.shape, out.dtype, np.abs(out).max())
